# revision 9
# baseline (speedup 1.0000x reference)
import sys
sys.path.insert(0, '/opt/trn_rl_repo')
import numpy as np
import concourse.bacc as bacc
import concourse.mybir as mybir
from concourse import bass_utils, tile

F32 = mybir.dt.float32
F32R = mybir.dt.float32r
RELU = mybir.ActivationFunctionType.Relu
IDENT = mybir.ActivationFunctionType.Identity
NCORES = 8
RGB_MEANS = np.array([104.008, 116.669, 122.675], np.float32).reshape(3, 1, 1)

_programs = {}


def emit_conv(nc, mp, wp, sc, ps, uname, x_blks, w_dram, b_dram, Cin, Cout,
              W, R, relu, y_blks=None, out_dram=None):
    """3x3 SAME conv on padded SBUF maps ([P, R+2, W+2] fp32r, zero pads).
    ib-outer loop with chunked PSUM tiles and rolling weight slots."""
    nbi = (Cin + 127) // 128
    nbo = (Cout + 127) // 128
    b_ts = []
    for ob in range(nbo):
        Ob = min(128, Cout - ob * 128)
        bt = wp.tile([Ob, 1], F32, tag=f"b_{uname}_{ob}")
        nc.sync.dma_start(bt[:], b_dram.ap()[ob * 128:ob * 128 + Ob, :])
        b_ts.append(bt)
    TR = min(max(1, 512 // W), R)
    tiles = []
    for r in range(0, R, TR):
        tr = min(TR, R - r)
        tr_mm = tr
        while tr_mm * W < 256 and tr_mm < R - r:
            tr_mm += 1
        tiles.append((r, tr, tr_mm))
    chunk = max(1, 8 // nbo)
    func = RELU if relu else IDENT
    for c0 in range(0, len(tiles), chunk):
        ch = tiles[c0:c0 + chunk]
        psums = {}
        for ti in range(len(ch)):
            for ob in range(nbo):
                pst = ps.tile([128, ch[ti][2] * W], F32, tag="ps")
                psums[(ti, ob)] = pst
        for ib in range(nbi):
            Ib = min(128, Cin - ib * 128)
            wt = wp.tile([128, 9 * Cout], F32R, tag="wsl")
            nc.gpsimd.dma_start(wt[:Ib, :],
                                w_dram.ap()[ib * 128:ib * 128 + Ib, :])
            for ti, (r, tr, tr_mm) in enumerate(ch):
                for ob in range(nbo):
                    Ob = min(128, Cout - ob * 128)
                    for t in range(9):
                        dy, dx = t // 3, t % 3
                        lhsT = wt[:Ib, t * Cout + ob * 128:
                                   t * Cout + ob * 128 + Ob]
                        rhs = x_blks[ib][:Ib, r + dy:r + dy + tr_mm, dx:dx + W]
                        nc.tensor.matmul(
                            psums[(ti, ob)][:Ob, :].rearrange(
                                "p (a b) -> p a b", a=tr_mm),
                            lhsT, rhs,
                            start=(ib == 0 and t == 0),
                            stop=(ib == nbi - 1 and t == 8))
        for ti, (r, tr, tr_mm) in enumerate(ch):
            for ob in range(nbo):
                Ob = min(128, Cout - ob * 128)
                pv = psums[(ti, ob)][:Ob, 0:tr * W].rearrange(
                    "p (a b) -> p a b", a=tr)
                bias = b_ts[ob][:Ob, 0:1]
                if out_dram is not None:
                    st = sc.tile([Cout, TR, W], F32, tag="stage")
                    nc.scalar.activation(st[ob * 128:ob * 128 + Ob, 0:tr, :],
                                         pv, func, bias=bias)
                    nc.sync.dma_start(out_dram.ap()[:, r:r + tr, :],
                                      st[:, 0:tr, :])
                else:
                    nc.scalar.activation(
                        y_blks[ob][:Ob, r + 1:r + 1 + tr, 1:W + 1],
                        pv, func, bias=bias)


def emit_pool(nc, x_blks, y_blks, C, W, R, sc):
    Rp, W2 = R // 2, W // 2
    rch = min(Rp, max(2, (2048 // W2) & ~1))
    for ib in range((C + 127) // 128):
        P = min(128, C - ib * 128)
        for r0 in range(0, Rp, rch):
            rc = min(rch, Rp - r0)
            tmp = sc.tile([128, 2 * rch, W2], F32R, tag="pooltmp")
            in1 = x_blks[ib][:P, 1 + 2 * r0:1 + 2 * r0 + 2 * rc, 1:W + 1]
            a = in1.rearrange("p r (c two) -> p r c two", two=2)
            nc.vector.tensor_max(tmp[:P, 0:2 * rc, :], a[:, :, :, 0],
                                 a[:, :, :, 1])
            b = tmp[:P, 0:2 * rc, :].rearrange(
                "p (j two) c -> p j two c", two=2)
            nc.vector.tensor_max(y_blks[ib][:P, r0 + 1:r0 + 1 + rc, 1:W2 + 1],
                                 b[:, :, 0, :], b[:, :, 1, :])


def new_map(nc, mp, C, R, W, tag):
    blks = []
    for ib in range((C + 127) // 128):
        P = min(128, C - ib * 128)
        t = mp.tile([128, R + 2, W + 2], F32R, tag=f"{tag}_{ib}")
        nc.gpsimd.memset(t[:].bitcast(F32), 0.0)
        blks.append(t)
    return blks


def conv_w_dram(nc, name, Cin, Cout):
    return (nc.dram_tensor(f"w_{name}", [Cin, 9 * Cout], F32,
                           kind="ExternalInput"),
            nc.dram_tensor(f"b_{name}", [Cout, 1], F32, kind="ExternalInput"))


def build_L1():
    nc = bacc.Bacc("TRN2", target_bir_lowering=False, debug=False,
                   num_devices=NCORES)
    x27 = nc.dram_tensor("x27", [27, 88, 512], F32, kind="ExternalInput")
    w11 = nc.dram_tensor("w_c11", [27, 64], F32, kind="ExternalInput")
    b11 = nc.dram_tensor("b_c11", [64, 1], F32, kind="ExternalInput")
    wbs = {}
    for nm, ci, co in (("c12", 64, 64), ("c21", 64, 128), ("c22", 128, 128),
                       ("d1", 128, 64), ("d2", 64, 32), ("d3", 32, 4)):
        wbs[nm] = conv_w_dram(nc, nm, ci, co)
    out3 = nc.dram_tensor("out3", [4, 44, 256], F32, kind="ExternalOutput")
    p2o = nc.dram_tensor("p2", [128, 22, 128], F32, kind="ExternalOutput")

    with tile.TileContext(nc) as tc:
        with (tc.tile_pool(name="mp", bufs=1) as mp,
              tc.tile_pool(name="wp", bufs=3) as wp,
              tc.tile_pool(name="sc", bufs=1) as sc,
              tc.tile_pool(name="ps", bufs=8, space="PSUM") as ps):
            p1 = new_map(nc, mp, 64, 44, 256, "p1")
            if True:
                bp = mp
                w11t = wp.tile([27, 64], F32R, tag="w11")
                nc.gpsimd.dma_start(w11t[:], w11.ap())
                b11t = wp.tile([64, 1], F32, tag="b11")
                nc.sync.dma_start(b11t[:], b11.ap())
                w12t = wp.tile([64, 9 * 64], F32R, tag="w12")
                nc.gpsimd.dma_start(w12t[:], wbs["c12"][0].ap())
                b12t = wp.tile([64, 1], F32, tag="b12")
                nc.sync.dma_start(b12t[:], wbs["c12"][1].ap())
                c11b = bp.tile([64, 12, 514], F32R, tag="mB_0")
                nc.gpsimd.memset(c11b[:].bitcast(F32), 0.0)
                c12b = bp.tile([64, 8, 512], F32R, tag="mA_0")
                for b in range(11):
                    if b == 0:
                        rows, s0 = (0, 10), 2
                    elif b == 10:
                        rows, s0 = (79, 88), 1
                        nc.gpsimd.memset(c11b[:, 10:12, :].bitcast(F32), 0.0)
                    else:
                        rows, s0 = (8 * b - 1, 8 * b + 9), 1
                    nrow = rows[1] - rows[0]
                    x27b = bp.tile([27, 10, 512], F32R, tag="p2m_0")
                    nc.gpsimd.dma_start(x27b[:, 0:nrow, :],
                                        x27.ap()[:, rows[0]:rows[1], :])
                    for j in range(nrow):
                        psum = ps.tile([64, 512], F32, tag="ps")
                        nc.tensor.matmul(psum[:], w11t[:], x27b[:, j, :],
                                         start=True, stop=True)
                        nc.scalar.activation(c11b[:, s0 + j, 1:513], psum[:],
                                             RELU, bias=b11t[:, 0:1])
                    for j in range(8):
                        psum = ps.tile([64, 512], F32, tag="ps")
                        for t in range(9):
                            dy, dx = t // 3, t % 3
                            nc.tensor.matmul(
                                psum[:], w12t[:, t * 64:(t + 1) * 64],
                                c11b[:, j + 1 + dy, dx:dx + 512],
                                start=(t == 0), stop=(t == 8))
                        nc.scalar.activation(c12b[:, j, :], psum[:],
                                             RELU, bias=b12t[:, 0:1])
                    tmp = sc.tile([64, 8, 256], F32R, tag="pooltmp1")
                    a = c12b[:].rearrange("p r (c two) -> p r c two", two=2)
                    nc.vector.tensor_max(tmp[:], a[:, :, :, 0], a[:, :, :, 1])
                    bb = tmp[:].rearrange("p (j two) c -> p j two c", two=2)
                    nc.vector.tensor_max(
                        p1[0][:64, 4 * b + 1:4 * b + 5, 1:257],
                        bb[:, :, 0, :], bb[:, :, 1, :])

            c21 = new_map(nc, mp, 128, 44, 256, "mB")
            emit_conv(nc, mp, wp, sc, ps, "c21", p1, *wbs["c21"],
                      64, 128, 256, 44, True, y_blks=c21)
            b2 = new_map(nc, mp, 128, 44, 256, "mA")
            emit_conv(nc, mp, wp, sc, ps, "c22", c21, *wbs["c22"],
                      128, 128, 256, 44, True, y_blks=b2)
            p2 = new_map(nc, mp, 128, 22, 128, "p2m")
            emit_pool(nc, b2, p2, 128, 256, 44, sc)
            nc.gpsimd.dma_start(p2o.ap(), p2[0][:, 1:23, 1:129])
            h1 = new_map(nc, mp, 64, 44, 256, "p1")
            emit_conv(nc, mp, wp, sc, ps, "d1", b2, *wbs["d1"],
                      128, 64, 256, 44, True, y_blks=h1)
            h2 = new_map(nc, mp, 32, 44, 256, "mB")
            emit_conv(nc, mp, wp, sc, ps, "d2", h1, *wbs["d2"],
                      64, 32, 256, 44, True, y_blks=h2)
            emit_conv(nc, mp, wp, sc, ps, "d3", h2, *wbs["d3"],
                      32, 4, 256, 44, False, out_dram=out3)
    nc.compile()
    return nc


def build_seg(in_C, in_R, in_W, convs, pool_C=None, exports=()):
    nc = bacc.Bacc("TRN2", target_bir_lowering=False, debug=False,
                   num_devices=NCORES)
    xin = nc.dram_tensor("xin", [in_C, in_R, in_W], F32, kind="ExternalInput")
    wds = {nm: conv_w_dram(nc, nm, ci, co) for nm, ci, co, _, _ in convs}
    outs = {nm: nc.dram_tensor(nm, [C, R, W], F32, kind="ExternalOutput")
            for nm, C, R, W in exports}
    with tile.TileContext(nc) as tc:
        with (tc.tile_pool(name="mp", bufs=1) as mp,
              tc.tile_pool(name="wp", bufs=3) as wp,
              tc.tile_pool(name="sc", bufs=1) as sc,
              tc.tile_pool(name="ps", bufs=8, space="PSUM") as ps):
            cur = new_map(nc, mp, in_C, in_R, in_W, "mA")
            for ib, t in enumerate(cur):
                P = min(128, in_C - ib * 128)
                nc.gpsimd.dma_start(t[:P, 1:in_R + 1, 1:in_W + 1],
                                    xin.ap()[ib * 128:ib * 128 + P, :, :])
            maps = {"xin": cur}
            tagflip = ["mB", "mA"]
            for i, (nm, Cin, Cout, relu, flat) in enumerate(convs):
                if flat:
                    emit_conv(nc, mp, wp, sc, ps, nm, cur, *wds[nm],
                              Cin, Cout, in_W, in_R, relu, out_dram=outs[nm])
                else:
                    nxt = new_map(nc, mp, Cout, in_R, in_W, tagflip[i % 2])
                    emit_conv(nc, mp, wp, sc, ps, nm, cur, *wds[nm],
                              Cin, Cout, in_W, in_R, relu, y_blks=nxt)
                    maps[nm] = nxt
                    cur = nxt
            if pool_C is not None:
                pmap = new_map(nc, mp, pool_C, in_R // 2, in_W // 2, "pool")
                emit_pool(nc, cur, pmap, pool_C, in_W, in_R, sc)
                maps["pool"] = pmap
            for nm, C, R, W in exports:
                src = maps.get(nm) or maps.get("pool" if nm.startswith("p") else nm)
                if src is None:
                    continue
                for ib in range((C + 127) // 128):
                    P = min(128, C - ib * 128)
                    nc.gpsimd.dma_start(
                        outs[nm].ap()[ib * 128:ib * 128 + P, :, :],
                        src[ib][:P, 1:R + 1, 1:W + 1])
    nc.compile()
    return nc


def get_programs():
    if not _programs:
        _programs["L1"] = build_L1()
        _programs["L2"] = build_seg(
            128, 24, 128,
            [("c31", 128, 256, True, False), ("c32", 256, 256, True, False),
             ("b3", 256, 256, True, False)],
            pool_C=256, exports=[("b3", 256, 24, 128), ("p3", 256, 12, 64)])
        _programs["L3a"] = build_seg(
            256, 26, 128,
            [("a1", 256, 256, True, False), ("a2", 256, 128, True, False),
             ("a3", 128, 64, True, False), ("a4", 64, 32, True, False),
             ("a5", 32, 4, False, True)],
            exports=[("a5", 4, 26, 128)])
        _programs["L3b"] = build_seg(
            256, 16, 64,
            [("c41", 256, 512, True, False), ("c42", 512, 512, True, False),
             ("b4", 512, 512, True, False)],
            pool_C=512, exports=[("b4", 512, 16, 64), ("p4", 512, 8, 32)])
        _programs["L4a"] = build_seg(
            512, 18, 64,
            [("hb1", 512, 256, True, False), ("hb2", 256, 128, True, False),
             ("hb3", 128, 64, True, False), ("hb4", 64, 32, True, False),
             ("hb5", 32, 4, False, True)],
            exports=[("hb5", 4, 18, 64)])
        _programs["L4b"] = build_seg(
            512, 10, 32,
            [("c51", 512, 512, True, False), ("c52", 512, 512, True, False),
             ("b5", 512, 512, True, False)],
            exports=[("b5", 512, 10, 32)])
        _programs["L5"] = build_seg(
            512, 14, 32,
            [("hc1", 512, 256, True, False), ("hc2", 256, 128, True, False),
             ("hc3", 128, 64, True, False), ("hc4", 64, 32, True, False),
             ("hc5", 32, 4, False, True)],
            exports=[("hc5", 4, 14, 32)])
    return _programs


def wpack(w):
    w = np.asarray(w)
    O, I = w.shape[0], w.shape[1]
    return np.ascontiguousarray(w.transpose(1, 2, 3, 0).reshape(I, 9 * O))


def bpack(b):
    return np.ascontiguousarray(np.asarray(b).reshape(-1, 1))


def gather_map(res, name, C, Hres, strip, row0s, R):
    full = None
    for c in range(NCORES):
        win = np.asarray(res.results[c][name])
        if full is None:
            full = np.zeros((C, Hres, win.shape[2]), np.float32)
        off = strip * c - row0s[c]
        full[:, strip * c:strip * (c + 1), :] = win[:, off:off + strip, :]
    return full


def windows(Hres, strip, o, R, even=False):
    r0s = []
    for c in range(NCORES):
        r0 = min(max(strip * c - o, 0), Hres - R)
        if even and r0 % 2:
            r0 += 1
        r0s.append(r0)
    return r0s


def _wcom(pairs):
    com = {}
    for nm, (w, b) in pairs:
        com[f"w_{nm}"] = wpack(w)
        com[f"b_{nm}"] = bpack(b)
    return com


def kernel(x, backbone, headA, headB, headC, headD, _trace=False, _times=None):
    progs = get_programs()
    x = np.asarray(x).reshape(3, 512, 512) - RGB_MEANS
    cores = list(range(NCORES))

    def run(key, in_maps):
        res = bass_utils.run_bass_kernel_spmd(progs[key], in_maps,
                                              core_ids=cores, trace=_trace)
        if _times is not None and res.exec_time_ns:
            _times.append((key, res.exec_time_ns))
        return res

    # ---- L1 ----
    r0_256 = windows(256, 32, 6, 44, even=True)
    w11 = np.asarray(backbone['conv1_1'][0])
    com = {"w_c11": np.ascontiguousarray(w11.reshape(64, 27).T),
           "b_c11": bpack(backbone['conv1_1'][1])}
    com.update(_wcom([("c12", backbone['conv1_2']), ("c21", backbone['conv2_1']),
                      ("c22", backbone['conv2_2']), ("d1", headD[0]),
                      ("d2", headD[1]), ("d3", headD[2])]))
    in_maps = []
    for c in range(NCORES):
        row0 = 2 * r0_256[c]
        xp = np.zeros((3, 90, 514), np.float32)
        lo, hi = max(0, row0 - 1), min(512, row0 + 89)
        xp[:, lo - (row0 - 1):hi - (row0 - 1), 1:513] = x[:, lo:hi, :]
        x27 = np.empty((27, 88, 512), np.float32)
        for ch in range(3):
            for dy in range(3):
                for dx in range(3):
                    x27[ch * 9 + 3 * dy + dx] = xp[ch, dy:dy + 88, dx:dx + 512]
        in_maps.append({"x27": x27, **com})
    res = run("L1", in_maps)
    out3 = gather_map(res, "out3", 4, 256, 32, r0_256, 44)
    p2 = gather_map(res, "p2", 128, 128, 16, [r // 2 for r in r0_256], 22)

    # ---- L2 ----
    r0_128 = windows(128, 16, 4, 24, even=True)
    com = _wcom([("c31", backbone['conv3_1']), ("c32", backbone['conv3_2']),
                 ("b3", backbone['conv3_3'])])
    res = run("L2", [{"xin": np.ascontiguousarray(p2[:, r0:r0 + 24, :]), **com}
                     for r0 in r0_128])
    b3 = gather_map(res, "b3", 256, 128, 16, r0_128, 24)
    p3 = gather_map(res, "p3", 256, 64, 8, [r // 2 for r in r0_128], 12)

    # ---- L3a ----
    r0a = windows(128, 16, 5, 26)
    com = _wcom([(nm, headA[i]) for i, nm in
                 enumerate(("a1", "a2", "a3", "a4", "a5"))])
    res = run("L3a", [{"xin": np.ascontiguousarray(b3[:, r0:r0 + 26, :]), **com}
                      for r0 in r0a])
    out2 = gather_map(res, "a5", 4, 128, 16, r0a, 26)

    # ---- L3b ----
    r0_64 = windows(64, 8, 4, 16, even=True)
    com = _wcom([("c41", backbone['conv4_1']), ("c42", backbone['conv4_2']),
                 ("b4", backbone['conv4_3'])])
    res = run("L3b", [{"xin": np.ascontiguousarray(p3[:, r0:r0 + 16, :]), **com}
                      for r0 in r0_64])
    b4 = gather_map(res, "b4", 512, 64, 8, r0_64, 16)
    p4 = gather_map(res, "p4", 512, 32, 4, [r // 2 for r in r0_64], 8)

    # ---- L4a ----
    r0b = windows(64, 8, 5, 18)
    com = _wcom([(nm, headB[i]) for i, nm in
                 enumerate(("hb1", "hb2", "hb3", "hb4", "hb5"))])
    res = run("L4a", [{"xin": np.ascontiguousarray(b4[:, r0:r0 + 18, :]), **com}
                      for r0 in r0b])
    out1 = gather_map(res, "hb5", 4, 64, 8, r0b, 18)

    # ---- L4b ----
    r0_32 = windows(32, 4, 3, 10)
    com = _wcom([("c51", backbone['conv5_1']), ("c52", backbone['conv5_2']),
                 ("b5", backbone['conv5_3'])])
    res = run("L4b", [{"xin": np.ascontiguousarray(p4[:, r0:r0 + 10, :]), **com}
                      for r0 in r0_32])
    b5 = gather_map(res, "b5", 512, 32, 4, r0_32, 10)

    # ---- L5 ----
    r0c = windows(32, 4, 5, 14)
    com = _wcom([(nm, headC[i]) for i, nm in
                 enumerate(("hc1", "hc2", "hc3", "hc4", "hc5"))])
    res = run("L5", [{"xin": np.ascontiguousarray(b5[:, r0:r0 + 14, :]), **com}
                     for r0 in r0c])
    out0 = gather_map(res, "hc5", 4, 32, 4, r0c, 14)

    return (out0[None].astype(np.float32), out1[None].astype(np.float32),
            out2[None].astype(np.float32), out3[None].astype(np.float32))
